# revision 2
# baseline (speedup 1.0000x reference)
"""Trainium2 Bass kernel for DPAttention (attention block + residual + LayerNorm).

Sharding: 8 cores = DP2 (batch) x TP4 (head groups of 3 heads).
Core c: b = c//4, g = c%4 -> heads [3g, 3g+3), output rows [512g, 512g+512) of batch b.

v2 design:
  - scores^T via ROW-TILED matmul pairs: two concurrent K=64 matmuls at
    tile_position (0,0)/(64,0) -> 2x PE throughput on scores.
    Pairs: p0=(h0,h1 | qh0), p1=(h2 qh0, h2 qh1), p2=(h0,h1 | qh1).
  - exp split across engines: unit-A tiles on ScalarE (exact exp, mask bias),
    a tunable share of unit-B tiles on VectorE via Schraudolph fast-exp
    (bf16 bits = int16(round(s * A/8 + (A*mask + B'))), saturates to -0 for
    masked keys).
  - kt-level software pipeline: ctx lags scores by LAG tiles; e lives in a
    4-slice ring per unit (not full [128,16,1024] tiles).
  - QKV+out-dense biases and projections upfront; ACT does proj bias adds.
  - ctx matmul M=65 (64 V dims + ones column -> softmax denominator).
  - 4-core AllGather of ctx^T per query half; dynamic-offset gather; out
    dense (bf16) + residual + LayerNorm.
"""
import numpy as np
import ml_dtypes

import concourse.bass as bass
import concourse.mybir as mybir
import concourse.tile as tile
from concourse import bacc
from concourse.bass_utils import run_bass_kernel_spmd

F32 = mybir.dt.float32
BF16 = mybir.dt.bfloat16
I16 = mybir.dt.int16
U32 = mybir.dt.uint32
AF = mybir.ActivationFunctionType
ALU = mybir.AluOpType

B, S, H, NH, HD = 2, 2048, 768, 12, 64
P = 128
KT = H // P            # 6 contraction tiles over hidden
ST = S // P            # 16 tiles over sequence
TP = 4                 # head groups (tensor-parallel within a batch)
HG = NH // TP          # 3 heads per core
HGD = HG * HD          # 192
SQ = S // TP           # 512 output rows per core
EPS = 1e-5
SCALE = 1.0 / np.sqrt(HD)
NCORES = 8
GROUPS = [[0, 1, 2, 3], [4, 5, 6, 7]]
BIGNEG = -1.0e9
BIGPOS = 1.0e18
LAG = 2                # ctx lags scores by LAG kt-tiles
VD = HD + 1            # ctx matmul M: 64 V dims + ones (denominator) column

# Schraudolph fast-exp constants (bf16 bits via int16 round, B calibrated)
SCH_A = 128.0 * 1.4426950408889634
SCH_B = 127.0 * 128.0 - 5.6
DVE_MOD = 4            # DVE takes unit-B exp tiles where kt % DVE_MOD != DVE_MOD-1

_cache = {}


def build():
    nc = bacc.Bacc(num_devices=NCORES)

    xt_d = nc.dram_tensor("xt", [H, S], BF16, kind="ExternalInput")
    xres_d = nc.dram_tensor("xres", [SQ, H], F32, kind="ExternalInput")
    wq_d = nc.dram_tensor("wq", [H, HGD], BF16, kind="ExternalInput")
    wk_d = nc.dram_tensor("wk", [H, HGD], BF16, kind="ExternalInput")
    wv_d = nc.dram_tensor("wv", [H, HGD], BF16, kind="ExternalInput")
    bq_d = nc.dram_tensor("bq", [HGD], F32, kind="ExternalInput")
    bk_d = nc.dram_tensor("bk", [HGD], F32, kind="ExternalInput")
    bvr_d = nc.dram_tensor("bvr", [P, HGD], F32, kind="ExternalInput")
    wo_d = nc.dram_tensor("wo", [H, H], BF16, kind="ExternalInput")
    mkb_d = nc.dram_tensor("mkb", [S], F32, kind="ExternalInput")
    mkb2_d = nc.dram_tensor("mkb2", [S], F32, kind="ExternalInput")
    gq_d = nc.dram_tensor("gq", [1, S], BF16, kind="ExternalInput")
    lng_d = nc.dram_tensor("lng", [P, H], F32, kind="ExternalInput")
    lnb_d = nc.dram_tensor("lnb", [P, H], F32, kind="ExternalInput")
    xsum_d = nc.dram_tensor("xsum", [SQ], F32, kind="ExternalInput")
    qoff_d = nc.dram_tensor("qoff", [1, 2], U32, kind="ExternalInput")
    out_d = nc.dram_tensor("out", [SQ, H], F32, kind="ExternalOutput")

    with tile.TileContext(nc) as tc:
        with (
            tc.tile_pool(name="wts", bufs=1) as wts,
            tc.tile_pool(name="qkv", bufs=1) as qkv,
            tc.tile_pool(name="dram", bufs=1, space="DRAM") as dram,
        ):
            # ---- load weights / small tensors ----
            wq_sb = wts.tile([P, KT, HGD], BF16)
            wk_sb = wts.tile([P, KT, HGD], BF16)
            wv_sb = wts.tile([P, KT, HGD], BF16)
            nc.sync.dma_start(wq_sb[:], wq_d.rearrange("(kt p) d -> p kt d", p=P))
            nc.sync.dma_start(wk_sb[:], wk_d.rearrange("(kt p) d -> p kt d", p=P))
            nc.sync.dma_start(wv_sb[:], wv_d.rearrange("(kt p) d -> p kt d", p=P))
            wo_sb = wts.tile([P, KT, H], BF16)
            nc.sync.dma_start(wo_sb[:], wo_d.rearrange("(kt p) n -> p kt n", p=P))

            bq_sb = wts.tile([P, 2], F32)
            bk_sb = wts.tile([P, 2], F32)
            nc.gpsimd.dma_start(bq_sb[:, 0:1], bq_d[0:P].rearrange("(p o) -> p o", o=1))
            nc.gpsimd.dma_start(bq_sb[0:HGD - P, 1:2], bq_d[P:HGD].rearrange("(p o) -> p o", o=1))
            nc.gpsimd.dma_start(bk_sb[:, 0:1], bk_d[0:P].rearrange("(p o) -> p o", o=1))
            nc.gpsimd.dma_start(bk_sb[0:HGD - P, 1:2], bk_d[P:HGD].rearrange("(p o) -> p o", o=1))
            bvr_sb = wts.tile([P, HG, HD], F32)
            nc.sync.dma_start(bvr_sb[:], bvr_d.rearrange("p (h d) -> p h d", d=HD))
            mkb_sb = wts.tile([P, ST], F32)
            nc.gpsimd.dma_start(mkb_sb[:], mkb_d.rearrange("(kt p) -> p kt", p=P))
            mkb2_sb = wts.tile([P, ST], F32)
            nc.gpsimd.dma_start(mkb2_sb[:], mkb2_d.rearrange("(kt p) -> p kt", p=P))
            gq_sb = wts.tile([1, S], BF16)
            nc.gpsimd.dma_start(gq_sb[:], gq_d[:])
            lng_sb = wts.tile([P, H], F32)
            lnb_sb = wts.tile([P, H], F32)
            nc.sync.dma_start(lng_sb[:], lng_d[:])
            nc.sync.dma_start(lnb_sb[:], lnb_d[:])
            xres_sb = wts.tile([P, SQ // P, H], F32)
            nc.sync.dma_start(xres_sb[:], xres_d.rearrange("(t p) n -> p t n", p=P))
            qoff_sb = wts.tile([1, 2], U32)
            nc.gpsimd.dma_start(qoff_sb[:], qoff_d[:])
            xsum_sb = wts.tile([P, SQ // P], F32)
            nc.gpsimd.dma_start(xsum_sb[:], xsum_d.rearrange("(t p) -> p t", p=P))

            ones_sb = wts.tile([P, 1], BF16)
            nc.gpsimd.memset(ones_sb[:], 1.0)

            # ---- persistent intermediate tiles ----
            qt_sb = qkv.tile([P, S], BF16)      # Q^T: h0 rows 0:64, h1 rows 64:128
            qt2_sb = qkv.tile([P, S], BF16)     # Q^T h2 duplicated in both halves
            ktp_sb = qkv.tile([P, 2, S], BF16)  # K^T slot0: h0+h1; slot1: h2 dup
            v_sb = qkv.tile([P, ST, HG, VD], BF16)  # V + ones col (denominator)
            u_sb = qkv.tile([1, HG, VD], BF16)      # mean_k V (+1 slot)
            ctxa_sb = qkv.tile([P, S], BF16)   # ctx^T heads 0,1
            ctxb_sb = qkv.tile([HD, S], BF16)  # ctx^T head 2

            nc.gpsimd.memset(v_sb[:, :, :, HD:VD], 1.0)

            xt_sb = qkv.tile([P, KT, S], BF16)
            xt_r = xt_d.rearrange("(kt p) s -> p kt s", p=P)
            for kt in range(KT):
                nc.sync.dma_start(xt_sb[:, kt, :], xt_r[:, kt, :])

            # ================= Q/K projections =================
            # Mpass0 -> heads 0,1 stacked on 128 partitions; Mpass1 -> head 2
            # (64 partitions, later duplicated into the upper half).
            with tc.tile_pool(name="pps", bufs=2, space="PSUM") as pps:
                for w_sb, b_sb, is_k in ((wq_sb, bq_sb, False), (wk_sb, bk_sb, True)):
                    for mp, (m0, msz) in enumerate(((0, P), (P, HGD - P))):
                        for qc in range(S // 512):
                            qs = slice(qc * 512, (qc + 1) * 512)
                            ps = pps.tile([P, 512], F32, tag="proj")
                            for kt in range(KT):
                                nc.tensor.matmul(
                                    ps[:msz],
                                    w_sb[:, kt, m0:m0 + msz],
                                    xt_sb[:, kt, qs],
                                    start=(kt == 0), stop=(kt == KT - 1),
                                )
                            if not is_k:
                                dst = qt_sb[:, qs] if mp == 0 else qt2_sb[0:64, qs]
                            else:
                                dst = (ktp_sb[:, 0, qs] if mp == 0
                                       else ktp_sb[0:64, 1, qs])
                            nc.scalar.activation(dst, ps[:msz], AF.Identity,
                                                 bias=b_sb[:msz, mp:mp + 1])
                # duplicate head-2 Q^T / K^T into partitions 64:128
                nc.sync.dma_start(qt2_sb[64:P, :], qt2_sb[0:64, :])
                nc.sync.dma_start(ktp_sb[64:P, 1, :], ktp_sb[0:64, 1, :])

            # ================= V projection + u =================
            with tc.tile_pool(name="vps", bufs=2, space="PSUM") as vps:
                for st in range(ST):
                    ps = vps.tile([P, HGD], F32, tag="v")
                    for kt in range(KT):
                        nc.tensor.matmul(
                            ps[:], xt_sb[:, kt, st * P:(st + 1) * P], wv_sb[:, kt, :],
                            start=(kt == 0), stop=(kt == KT - 1),
                        )
                    nc.vector.tensor_tensor(
                        v_sb[:, st, :, 0:HD], ps[:].rearrange("p (h d) -> p h d", d=HD),
                        bvr_sb[:], op=ALU.add,
                    )
                ups = vps.tile([1, HGD], F32, tag="v", name="ups")
                for st in range(ST):
                    nc.tensor.matmul(
                        ups[:], ones_sb[:], v_sb[:, st, :, 0:HD],
                        start=(st == 0), stop=(st == ST - 1),
                    )
                nc.vector.tensor_scalar_mul(
                    u_sb[0:1, :, 0:HD],
                    ups[:].rearrange("p (h d) -> p h d", d=HD), 1.0 / S)
                nc.gpsimd.memset(u_sb[:, :, HD:VD], 1.0)

            # ================= attention =================
            # pairs: (K slot, qh of unit A, qh of unit B, head of A, head of B)
            pairs = [
                (0, 0, 0, 0, 1),   # h0 qh0 / h1 qh0
                (1, 0, 1, 2, 2),   # h2 qh0 / h2 qh1
                (0, 1, 1, 0, 1),   # h0 qh1 / h1 qh1
            ]

            from contextlib import ExitStack
            attn_ctx = ExitStack()
            epool = attn_ctx.enter_context(tc.tile_pool(name="epool", bufs=2))
            sps = attn_ctx.enter_context(tc.tile_pool(name="sps", bufs=1, space="PSUM"))
            cps = attn_ctx.enter_context(tc.tile_pool(name="cps", bufs=1, space="PSUM"))
            npool = attn_ctx.enter_context(tc.tile_pool(name="npool", bufs=2))

            ag_in = [dram.tile([HGD, 1024], BF16, name="agi0"),
                     dram.tile([HGD, 1024], BF16, name="agi1")]
            ag_out = dram.tile([2, TP, HGD, 1024], BF16)

            def emit_ag(qh):
                q0 = qh * 1024
                nc.sync.dma_start(ag_in[qh][0:P, :], ctxa_sb[:, q0:q0 + 1024])
                nc.sync.dma_start(ag_in[qh][P:HGD, :], ctxb_sb[:, q0:q0 + 1024])
                nc.gpsimd.collective_compute(
                    "AllGather", ALU.bypass, replica_groups=GROUPS,
                    ins=[ag_in[qh].opt()], outs=[ag_out[qh].opt()],
                )

            for ip, (slot, qhA, qhB, hA, hB) in enumerate(pairs):
                eA = epool.tile([P, 4, 1024], BF16, tag="eA", name=f"eA{ip}")
                eB = epool.tile([P, 4, 1024], BF16, tag="eB", name=f"eB{ip}")
                pcA = cps.tile([P, 1024], F32, tag="cA", name=f"cA{ip}")
                pcB = cps.tile([P, 1024], F32, tag="cB", name=f"cB{ip}")
                qsrc = qt_sb if slot == 0 else qt2_sb

                for step in range(ST + LAG):
                    if step < ST:
                        kt = step
                        ks = slice(kt * P, (kt + 1) * P)
                        psA = sps.tile([P, 1024], F32, tag="sA", name=f"sA{ip}_{kt}")
                        psB = sps.tile([P, 1024], F32, tag="sB", name=f"sB{ip}_{kt}")
                        for sub in range(2):
                            qA = qhA * 1024 + sub * 512
                            qB = qhB * 1024 + sub * 512
                            ss = slice(sub * 512, (sub + 1) * 512)
                            nc.tensor.matmul(psA[:, ss], ktp_sb[0:64, slot, ks],
                                             qsrc[0:64, qA:qA + 512],
                                             start=True, stop=True)
                            nc.tensor.matmul(psB[:, ss], ktp_sb[64:P, slot, ks],
                                             qsrc[64:P, qB:qB + 512],
                                             start=True, stop=True)
                        nc.scalar.activation(eA[:, kt % 4, :], psA[:], AF.Exp,
                                             bias=mkb_sb[:, kt:kt + 1],
                                             scale=float(SCALE))
                        if kt % DVE_MOD != DVE_MOD - 1:
                            nc.vector.tensor_scalar(
                                eB[:, kt % 4, :].bitcast(I16), psB[:],
                                float(SCH_A * SCALE), mkb2_sb[:, kt:kt + 1],
                                ALU.mult, ALU.add)
                        else:
                            nc.scalar.activation(eB[:, kt % 4, :], psB[:], AF.Exp,
                                                 bias=mkb_sb[:, kt:kt + 1],
                                                 scale=float(SCALE))
                    if step >= LAG:
                        kt2 = step - LAG
                        for sub in range(2):
                            ss = slice(sub * 512, (sub + 1) * 512)
                            nc.tensor.matmul(
                                pcA[0:VD, ss], v_sb[:, kt2, hA, :],
                                eA[:, kt2 % 4, ss],
                                start=(kt2 == 0), stop=False)
                            nc.tensor.matmul(
                                pcB[0:VD, ss], v_sb[:, kt2, hB, :],
                                eB[:, kt2 % 4, ss],
                                start=(kt2 == 0), stop=False)

                # ---- pair tail: invalid-query fix, denominator, normalize ----
                for unit, (pc, h, qh) in enumerate(
                        ((pcA, hA, qhA), (pcB, hB, qhB))):
                    q0 = qh * 1024
                    for sub in range(2):
                        ss = slice(sub * 512, (sub + 1) * 512)
                        nc.tensor.matmul(pc[0:VD, ss], u_sb[0:1, h, :],
                                         gq_sb[0:1, q0 + sub * 512:q0 + (sub + 1) * 512],
                                         start=False, stop=True)
                    rden = dram.tile([1, 1024], F32, tag="rden", bufs=2,
                                     name=f"rden{ip}_{unit}")
                    den = npool.tile([1, 1024], F32, tag="den")
                    nc.vector.tensor_copy(den[:], pc[HD:HD + 1, :])
                    nc.vector.reciprocal_approx_fast(den[:], den[:])
                    nc.sync.dma_start(rden[:], den[:])
                    rb = npool.tile([HD, 1024], F32, tag="rb")
                    nc.sync.dma_start(rb[:], rden[0:1, :].to_broadcast((HD, 1024)))
                    if slot == 0:
                        dst = ctxa_sb[HD * unit:HD * (unit + 1), q0:q0 + 1024]
                    else:
                        dst = ctxb_sb[:, q0:q0 + 1024]
                    nc.vector.tensor_tensor(dst, pc[0:HD, :], rb[:], op=ALU.mult)

                if ip == 1:
                    emit_ag(0)
                elif ip == 2:
                    emit_ag(1)
            attn_ctx.close()

            # ================= gather ctx across the TP group =================
            with tc.tile_critical():
                with nc.gpsimd.register("qx") as qx_reg:
                    nc.gpsimd.reg_load(qx_reg, qoff_sb[0:1, 0:1])
                    qx_v = nc.gpsimd.snap(qx_reg)
                with nc.gpsimd.register("qi") as qi_reg:
                    nc.gpsimd.reg_load(qi_reg, qoff_sb[0:1, 1:2])
                    qi_v = nc.gpsimd.snap(qi_reg)

            ctxg_sb = qkv.tile([P, KT, SQ], BF16)
            nc.gpsimd.dma_start(
                ctxg_sb[:],
                ag_out.rearrange("x g d q -> x (g d) q")
                .rearrange("x (kt p) q -> p kt x q", p=P)[
                    :, :, bass.ds(qx_v, 1), bass.ds(qi_v, SQ)],
            )

            # ================= out dense + residual + LayerNorm =================
            NST = SQ // P
            with tc.tile_pool(name="ops", bufs=2, space="PSUM") as ops, \
                 tc.tile_pool(name="lnp", bufs=1) as lnp:
                h_all = lnp.tile([P, NST, H], F32)
                mu_all = lnp.tile([P, NST], F32)
                var_all = lnp.tile([P, NST], F32)
                sq_tmp = lnp.tile([P, H], F32, bufs=2)
                for st4 in range(NST):
                    ps = ops.tile([P, H], F32, tag="od", name=f"od{st4}")
                    for kt in range(KT):
                        lhsT = ctxg_sb[:, kt, st4 * P:(st4 + 1) * P]
                        nc.tensor.matmul(ps[:, 0:512], lhsT, wo_sb[:, kt, 0:512],
                                         start=(kt == 0), stop=(kt == KT - 1))
                        nc.tensor.matmul(ps[:, 512:H], lhsT, wo_sb[:, kt, 512:H],
                                         start=(kt == 0), stop=(kt == KT - 1))
                    # h = out_dense + (x + bo); mean via ACT copy-accumulate
                    psc = lnp.tile([P, H], F32, tag="psc", bufs=2, name=f"psc{st4}")
                    nc.scalar.activation(psc[:], ps[:], AF.Identity,
                                         accum_out=mu_all[:, st4:st4 + 1])
                    nc.vector.tensor_tensor(h_all[:, st4, :], psc[:],
                                            xres_sb[:, st4, :], op=ALU.add)

                # mu = (psum_rowsum + xres_rowsum)/H ; var via ACT Square-accum
                nc.vector.tensor_tensor(mu_all[:], mu_all[:], xsum_sb[:], op=ALU.add)
                nc.vector.tensor_scalar_mul(mu_all[:], mu_all[:], 1.0 / H)
                negmu = lnp.tile([P, NST], F32)
                nc.vector.tensor_scalar_mul(negmu[:], mu_all[:], -1.0)
                for st4 in range(NST):
                    nc.scalar.activation(sq_tmp[:], h_all[:, st4, :], AF.Square,
                                         bias=negmu[:, st4:st4 + 1],
                                         accum_out=var_all[:, st4:st4 + 1])
                nc.vector.tensor_scalar_mul(var_all[:], var_all[:], 1.0 / H)
                nc.vector.tensor_scalar_add(var_all[:], var_all[:], EPS)
                # rstd = 1/sqrt(var) with one Newton step (batched over stiles)
                std0 = lnp.tile([P, NST], F32)
                nc.scalar.activation(std0[:], var_all[:], AF.Sqrt)
                y0 = lnp.tile([P, NST], F32)
                nc.vector.reciprocal(y0[:], std0[:])
                t0 = lnp.tile([P, NST], F32)
                nc.vector.tensor_tensor(t0[:], y0[:], y0[:], op=ALU.mult)
                nc.vector.tensor_tensor(t0[:], t0[:], var_all[:], op=ALU.mult)
                nc.vector.tensor_scalar_mul(t0[:], t0[:], -0.5)
                nc.vector.tensor_scalar_add(t0[:], t0[:], 1.5)
                rstd = lnp.tile([P, NST], F32)
                nc.vector.tensor_tensor(rstd[:], y0[:], t0[:], op=ALU.mult)

                for st4 in range(NST):
                    hc = lnp.tile([P, H], F32, tag="hc", bufs=2, name=f"hc{st4}")
                    nc.vector.tensor_scalar_sub(hc[:], h_all[:, st4, :],
                                                mu_all[:, st4:st4 + 1])
                    o_sb = lnp.tile([P, H], F32, tag="o", bufs=2, name=f"o{st4}")
                    nc.vector.scalar_tensor_tensor(
                        out=o_sb[:], in0=hc[:], scalar=rstd[:, st4:st4 + 1],
                        in1=lng_sb[:], op0=ALU.mult, op1=ALU.mult)
                    nc.vector.tensor_tensor(o_sb[:], o_sb[:], lnb_sb[:], op=ALU.add)
                    nc.sync.dma_start(out_d[st4 * P:(st4 + 1) * P, :], o_sb[:])

    nc.compile()
    return nc


def _prep_inputs(inputs):
    hs = np.asarray(inputs["hidden_states"], dtype=np.float32)
    am = np.asarray(inputs["attention_mask"], dtype=np.float32)
    Wq = np.asarray(inputs["Wq"], dtype=np.float32)
    Wk = np.asarray(inputs["Wk"], dtype=np.float32)
    Wv = np.asarray(inputs["Wv"], dtype=np.float32)
    Wo = np.asarray(inputs["Wo"], dtype=np.float32)
    bq = np.asarray(inputs["bq"], dtype=np.float32)
    bk = np.asarray(inputs["bk"], dtype=np.float32)
    bv = np.asarray(inputs["bv"], dtype=np.float32)
    bo = np.asarray(inputs["bo"], dtype=np.float32)
    lng = np.asarray(inputs["ln_gamma"], dtype=np.float32)
    lnb = np.asarray(inputs["ln_beta"], dtype=np.float32)

    wo_bf = Wo.astype(ml_dtypes.bfloat16)
    lng_rep = np.ascontiguousarray(np.broadcast_to(lng, (P, H)))
    lnb_rep = np.ascontiguousarray(np.broadcast_to(lnb, (P, H)))

    in_maps = []
    for c in range(NCORES):
        b, g = c // TP, c % TP
        cs = slice(HGD * g, HGD * (g + 1))
        mk = np.where(am[b] >= 0, 0.0, BIGNEG).astype(np.float32)
        mk2 = np.where(am[b] >= 0, np.float32(SCH_B),
                       np.float32(SCH_A * BIGNEG)).astype(np.float32)
        gqv = np.where(am[b] >= 0, 0.0, BIGPOS).astype(ml_dtypes.bfloat16)[None, :]
        in_maps.append({
            "xt": np.ascontiguousarray(hs[b].T).astype(ml_dtypes.bfloat16),
            "xres": np.ascontiguousarray(hs[b, SQ * g:SQ * (g + 1)] + bo),
            "wq": np.ascontiguousarray(Wq[:, cs]).astype(ml_dtypes.bfloat16),
            "wk": np.ascontiguousarray(Wk[:, cs]).astype(ml_dtypes.bfloat16),
            "wv": np.ascontiguousarray(Wv[:, cs]).astype(ml_dtypes.bfloat16),
            "bq": np.ascontiguousarray(bq[cs]),
            "bk": np.ascontiguousarray(bk[cs]),
            "bvr": np.ascontiguousarray(np.broadcast_to(bv[cs], (P, HGD))),
            "wo": np.ascontiguousarray(wo_bf),
            "mkb": mk,
            "mkb2": mk2,
            "gq": np.ascontiguousarray(gqv),
            "lng": lng_rep,
            "lnb": lnb_rep,
            "xsum": np.ascontiguousarray(
                (hs[b, SQ * g:SQ * (g + 1)] + bo).sum(axis=1).astype(np.float32)),
            "qoff": np.array([[g // 2, (g % 2) * SQ]], dtype=np.uint32),
        })
    return in_maps


def _run(inputs, trace=False, trace_cores=None):
    if "nc" not in _cache:
        _cache["nc"] = build()
    nc = _cache["nc"]
    in_maps = _prep_inputs(inputs)
    res = run_bass_kernel_spmd(
        nc, in_maps, list(range(NCORES)), trace=trace,
        trace_cores=trace_cores,
    )
    out = np.empty((B, S, H), dtype=np.float32)
    for c in range(NCORES):
        b, g = c // TP, c % TP
        out[b, SQ * g:SQ * (g + 1)] = res.results[c]["out"]
    return out, res


def kernel(**inputs) -> np.ndarray:
    out, _ = _run(inputs)
    return out


# revision 6
# speedup vs baseline: 1.0941x; 1.0941x over previous
"""Trainium2 Bass kernel for DPAttention (attention block + residual + LayerNorm).

Sharding: 8 cores = DP2 (batch) x TP4 (head groups of 3 heads).
Core c: b = c//4, g = c%4 -> heads [3g, 3g+3), output rows [512g, 512g+512) of batch b.

v2 design:
  - scores^T via ROW-TILED matmul pairs: two concurrent K=64 matmuls at
    tile_position (0,0)/(64,0) -> 2x PE throughput on scores.
    Pairs: p0=(h0,h1 | qh0), p1=(h2 qh0, h2 qh1), p2=(h0,h1 | qh1).
  - exp split across engines: unit-A tiles on ScalarE (exact exp, mask bias),
    a tunable share of unit-B tiles on VectorE via Schraudolph fast-exp
    (bf16 bits = int16(round(s * A/8 + (A*mask + B'))), saturates to -0 for
    masked keys).
  - kt-level software pipeline: ctx lags scores by LAG tiles; e lives in a
    4-slice ring per unit (not full [128,16,1024] tiles).
  - QKV+out-dense biases and projections upfront; ACT does proj bias adds.
  - ctx matmul M=65 (64 V dims + ones column -> softmax denominator).
  - 4-core AllGather of ctx^T per query half; dynamic-offset gather; out
    dense (bf16) + residual + LayerNorm.
"""
import numpy as np
import ml_dtypes

import concourse.bass as bass
import concourse.mybir as mybir
import concourse.tile as tile
from concourse import bacc
from concourse.bass_utils import run_bass_kernel_spmd

F32 = mybir.dt.float32
BF16 = mybir.dt.bfloat16
FP8 = mybir.dt.float8e4
I16 = mybir.dt.int16
U32 = mybir.dt.uint32
AF = mybir.ActivationFunctionType
ALU = mybir.AluOpType

B, S, H, NH, HD = 2, 2048, 768, 12, 64
P = 128
KT = H // P            # 6 contraction tiles over hidden
ST = S // P            # 16 tiles over sequence
TP = 4                 # head groups (tensor-parallel within a batch)
HG = NH // TP          # 3 heads per core
HGD = HG * HD          # 192
SQ = S // TP           # 512 output rows per core
EPS = 1e-5
SCALE = 1.0 / np.sqrt(HD)
NCORES = 8
GROUPS = [[0, 1, 2, 3], [4, 5, 6, 7]]
BIGNEG = -1.0e9
BIGPOS = 1.0e18
LAG = 2                # ctx lags scores by LAG kt-tiles
VD = HD + 1            # ctx matmul M: 64 V dims + ones (denominator) column

# Schraudolph fast-exp constants (bf16 bits via int16 round, B calibrated)
SCH_A = 128.0 * 1.4426950408889634
SCH_B = 127.0 * 128.0 - 5.6
DVE_MOD = 4            # DVE takes unit-B exp tiles where kt % DVE_MOD != DVE_MOD-1

_cache = {}


def build():
    nc = bacc.Bacc(num_devices=NCORES)

    xt_d = nc.dram_tensor("xt", [H, S], BF16, kind="ExternalInput")
    xres_d = nc.dram_tensor("xres", [SQ, H], F32, kind="ExternalInput")
    wq_d = nc.dram_tensor("wq", [H, HGD], BF16, kind="ExternalInput")
    wk_d = nc.dram_tensor("wk", [H, HGD], BF16, kind="ExternalInput")
    wv_d = nc.dram_tensor("wv", [H, HGD], BF16, kind="ExternalInput")
    bq_d = nc.dram_tensor("bq", [HGD], F32, kind="ExternalInput")
    bk_d = nc.dram_tensor("bk", [HGD], F32, kind="ExternalInput")
    bvr_d = nc.dram_tensor("bvr", [P, HGD], F32, kind="ExternalInput")
    wo_d = nc.dram_tensor("wo", [H, H], BF16, kind="ExternalInput")
    mkb_d = nc.dram_tensor("mkb", [S], F32, kind="ExternalInput")
    mkb2_d = nc.dram_tensor("mkb2", [S], F32, kind="ExternalInput")
    gq_d = nc.dram_tensor("gq", [1, S], BF16, kind="ExternalInput")
    lng_d = nc.dram_tensor("lng", [P, H], F32, kind="ExternalInput")
    lnb_d = nc.dram_tensor("lnb", [P, H], F32, kind="ExternalInput")
    xsum_d = nc.dram_tensor("xsum", [SQ], F32, kind="ExternalInput")
    qoff_d = nc.dram_tensor("qoff", [1, 1], U32, kind="ExternalInput")
    out_d = nc.dram_tensor("out", [SQ, H], F32, kind="ExternalOutput")

    with tile.TileContext(nc) as tc:
        with (
            tc.tile_pool(name="wts", bufs=1) as wts,
            tc.tile_pool(name="qkv", bufs=1) as qkv,
            tc.tile_pool(name="dram", bufs=1, space="DRAM") as dram,
        ):
            # ---- load weights / small tensors ----
            wq_sb = wts.tile([P, KT, HGD], BF16)
            wk_sb = wts.tile([P, KT, HGD], BF16)
            wv_sb = wts.tile([P, KT, HGD], BF16)
            nc.sync.dma_start(wq_sb[:], wq_d.rearrange("(kt p) d -> p kt d", p=P))
            nc.sync.dma_start(wk_sb[:], wk_d.rearrange("(kt p) d -> p kt d", p=P))
            nc.sync.dma_start(wv_sb[:], wv_d.rearrange("(kt p) d -> p kt d", p=P))
            wo_sb = wts.tile([P, KT, H], BF16)
            nc.sync.dma_start(wo_sb[:], wo_d.rearrange("(kt p) n -> p kt n", p=P))

            bq_sb = wts.tile([P, 2], F32)
            bk_sb = wts.tile([P, 2], F32)
            nc.gpsimd.dma_start(bq_sb[:, 0:1], bq_d[0:P].rearrange("(p o) -> p o", o=1))
            nc.gpsimd.dma_start(bq_sb[0:HGD - P, 1:2], bq_d[P:HGD].rearrange("(p o) -> p o", o=1))
            nc.gpsimd.dma_start(bk_sb[:, 0:1], bk_d[0:P].rearrange("(p o) -> p o", o=1))
            nc.gpsimd.dma_start(bk_sb[0:HGD - P, 1:2], bk_d[P:HGD].rearrange("(p o) -> p o", o=1))
            bvr_sb = wts.tile([P, HG, HD], F32)
            nc.sync.dma_start(bvr_sb[:], bvr_d.rearrange("p (h d) -> p h d", d=HD))
            mkb_sb = wts.tile([P, ST], F32)
            nc.gpsimd.dma_start(mkb_sb[:], mkb_d.rearrange("(kt p) -> p kt", p=P))
            mkb2_sb = wts.tile([P, ST], F32)
            nc.gpsimd.dma_start(mkb2_sb[:], mkb2_d.rearrange("(kt p) -> p kt", p=P))
            gq_sb = wts.tile([1, S], BF16)
            nc.gpsimd.dma_start(gq_sb[:], gq_d[:])
            lng_sb = wts.tile([P, H], F32)
            lnb_sb = wts.tile([P, H], F32)
            nc.sync.dma_start(lng_sb[:], lng_d[:])
            nc.sync.dma_start(lnb_sb[:], lnb_d[:])
            xres_sb = wts.tile([P, SQ // P, H], F32)
            nc.sync.dma_start(xres_sb[:], xres_d.rearrange("(t p) n -> p t n", p=P))
            xsum_sb = wts.tile([P, SQ // P], F32)
            nc.gpsimd.dma_start(xsum_sb[:], xsum_d.rearrange("(t p) -> p t", p=P))
            qoff_sb = wts.tile([1, 1], U32)
            nc.gpsimd.dma_start(qoff_sb[:], qoff_d[:])

            ones_sb = wts.tile([P, 1], BF16)
            nc.gpsimd.memset(ones_sb[:], 1.0)

            # ---- persistent intermediate tiles ----
            qt_sb = qkv.tile([P, S], BF16)      # Q^T: h0 rows 0:64, h1 rows 64:128
            qt2_sb = qkv.tile([P, S], BF16)     # Q^T h2 duplicated in both halves
            ktp_sb = qkv.tile([P, 2, S], BF16)  # K^T slot0: h0+h1; slot1: h2 dup
            v_sb = qkv.tile([P, ST, HG, VD], BF16)  # V + ones col (denominator)
            u_sb = qkv.tile([1, HG, VD], BF16)      # mean_k V (+1 slot)
            ctxa_sb = qkv.tile([P, S], FP8)   # ctx^T heads 0,1 (fp8 for A2A)
            ctxb_sb = qkv.tile([HD, S], FP8)  # ctx^T head 2

            nc.gpsimd.memset(v_sb[:, :, :, HD:VD], 1.0)

            xt_sb = qkv.tile([P, KT, S], BF16)
            xt_r = xt_d.rearrange("(kt p) s -> p kt s", p=P)
            for kt in range(KT):
                nc.sync.dma_start(xt_sb[:, kt, :], xt_r[:, kt, :])

            # ================= Q/K projections =================
            # Mpass0 -> heads 0,1 stacked on 128 partitions; Mpass1 -> head 2
            # (64 partitions, later duplicated into the upper half).
            with tc.tile_pool(name="pps", bufs=2, space="PSUM") as pps:
                for w_sb, b_sb, is_k in ((wq_sb, bq_sb, False), (wk_sb, bk_sb, True)):
                    for mp, (m0, msz) in enumerate(((0, P), (P, HGD - P))):
                        for qc in range(S // 512):
                            qs = slice(qc * 512, (qc + 1) * 512)
                            ps = pps.tile([P, 512], F32, tag="proj")
                            for kt in range(KT):
                                nc.tensor.matmul(
                                    ps[:msz],
                                    w_sb[:, kt, m0:m0 + msz],
                                    xt_sb[:, kt, qs],
                                    start=(kt == 0), stop=(kt == KT - 1),
                                )
                            if not is_k:
                                dst = qt_sb[:, qs] if mp == 0 else qt2_sb[0:64, qs]
                            else:
                                dst = (ktp_sb[:, 0, qs] if mp == 0
                                       else ktp_sb[0:64, 1, qs])
                            nc.scalar.activation(dst, ps[:msz], AF.Identity,
                                                 bias=b_sb[:msz, mp:mp + 1])
                # duplicate head-2 Q^T / K^T into partitions 64:128
                nc.sync.dma_start(qt2_sb[64:P, :], qt2_sb[0:64, :])
                nc.sync.dma_start(ktp_sb[64:P, 1, :], ktp_sb[0:64, 1, :])

            # ================= V projection + u =================
            with tc.tile_pool(name="vps", bufs=2, space="PSUM") as vps:
                for st in range(ST):
                    ps = vps.tile([P, HGD], F32, tag="v")
                    for kt in range(KT):
                        nc.tensor.matmul(
                            ps[:], xt_sb[:, kt, st * P:(st + 1) * P], wv_sb[:, kt, :],
                            start=(kt == 0), stop=(kt == KT - 1),
                        )
                    nc.vector.tensor_tensor(
                        v_sb[:, st, :, 0:HD], ps[:].rearrange("p (h d) -> p h d", d=HD),
                        bvr_sb[:], op=ALU.add,
                    )
                ups = vps.tile([1, HGD], F32, tag="v", name="ups")
                for st in range(ST):
                    nc.tensor.matmul(
                        ups[:], ones_sb[:], v_sb[:, st, :, 0:HD],
                        start=(st == 0), stop=(st == ST - 1),
                    )
                nc.vector.tensor_scalar_mul(
                    u_sb[0:1, :, 0:HD],
                    ups[:].rearrange("p (h d) -> p h d", d=HD), 1.0 / S)
                nc.gpsimd.memset(u_sb[:, :, HD:VD], 1.0)

            # ================= attention =================
            # pairs: (K slot, qh of unit A, qh of unit B, head of A, head of B)
            pairs = [
                (0, 0, 0, 0, 1),   # h0 qh0 / h1 qh0
                (1, 0, 1, 2, 2),   # h2 qh0 / h2 qh1
                (0, 1, 1, 0, 1),   # h0 qh1 / h1 qh1
            ]

            from contextlib import ExitStack
            attn_ctx = ExitStack()
            epool = attn_ctx.enter_context(tc.tile_pool(name="epool", bufs=2))
            sps = attn_ctx.enter_context(tc.tile_pool(name="sps", bufs=1, space="PSUM"))
            cps = attn_ctx.enter_context(tc.tile_pool(name="cps", bufs=1, space="PSUM"))
            npool = attn_ctx.enter_context(tc.tile_pool(name="npool", bufs=2))

            # 8-rank AllToAll (4-rank mesh unsupported): shard j carries ctx^T
            # columns for query block j%4; each core keeps the 4 slots of its
            # batch group. fp8 shards halve the wire bytes.
            a2a_in = dram.tile([2 * TP, HGD, 512], FP8, name="a2ai")
            a2a_out = dram.tile([2 * TP, HGD, 512], FP8, name="a2ao")

            def emit_shard(m):
                q0 = m * 512
                for j in (m, m + TP):
                    nc.sync.dma_start(a2a_in[j, 0:P, :], ctxa_sb[:, q0:q0 + 512])
                    nc.sync.dma_start(a2a_in[j, P:HGD, :], ctxb_sb[:, q0:q0 + 512])

            for ip, (slot, qhA, qhB, hA, hB) in enumerate(pairs):
                eA = epool.tile([P, 4, 1024], BF16, tag="eA", name=f"eA{ip}")
                eB = epool.tile([P, 4, 1024], BF16, tag="eB", name=f"eB{ip}")
                pcA = cps.tile([P, 1024], F32, tag="cA", name=f"cA{ip}")
                pcB = cps.tile([P, 1024], F32, tag="cB", name=f"cB{ip}")
                qsrc = qt_sb if slot == 0 else qt2_sb

                for step in range(ST + LAG):
                    if step < ST:
                        kt = step
                        ks = slice(kt * P, (kt + 1) * P)
                        psA = sps.tile([P, 1024], F32, tag="sA", name=f"sA{ip}_{kt}")
                        psB = sps.tile([P, 1024], F32, tag="sB", name=f"sB{ip}_{kt}")
                        for sub in range(2):
                            qA = qhA * 1024 + sub * 512
                            qB = qhB * 1024 + sub * 512
                            ss = slice(sub * 512, (sub + 1) * 512)
                            nc.tensor.matmul(psA[:, ss], ktp_sb[0:64, slot, ks],
                                             qsrc[0:64, qA:qA + 512],
                                             start=True, stop=True)
                            nc.tensor.matmul(psB[:, ss], ktp_sb[64:P, slot, ks],
                                             qsrc[64:P, qB:qB + 512],
                                             start=True, stop=True)
                        nc.scalar.activation(eA[:, kt % 4, :], psA[:], AF.Exp,
                                             bias=mkb_sb[:, kt:kt + 1],
                                             scale=float(SCALE))
                        if kt % DVE_MOD != DVE_MOD - 1:
                            nc.vector.tensor_scalar(
                                eB[:, kt % 4, :].bitcast(I16), psB[:],
                                float(SCH_A * SCALE), mkb2_sb[:, kt:kt + 1],
                                ALU.mult, ALU.add)
                        else:
                            nc.scalar.activation(eB[:, kt % 4, :], psB[:], AF.Exp,
                                                 bias=mkb_sb[:, kt:kt + 1],
                                                 scale=float(SCALE))
                    if step >= LAG:
                        kt2 = step - LAG
                        for sub in range(2):
                            ss = slice(sub * 512, (sub + 1) * 512)
                            nc.tensor.matmul(
                                pcA[0:VD, ss], v_sb[:, kt2, hA, :],
                                eA[:, kt2 % 4, ss],
                                start=(kt2 == 0), stop=False)
                            nc.tensor.matmul(
                                pcB[0:VD, ss], v_sb[:, kt2, hB, :],
                                eB[:, kt2 % 4, ss],
                                start=(kt2 == 0), stop=False)

                # ---- pair tail: invalid-query fix, denominator, normalize ----
                for unit, (pc, h, qh) in enumerate(
                        ((pcA, hA, qhA), (pcB, hB, qhB))):
                    q0 = qh * 1024
                    for sub in range(2):
                        ss = slice(sub * 512, (sub + 1) * 512)
                        nc.tensor.matmul(pc[0:VD, ss], u_sb[0:1, h, :],
                                         gq_sb[0:1, q0 + sub * 512:q0 + (sub + 1) * 512],
                                         start=False, stop=True)
                    rden = dram.tile([1, 1024], F32, tag="rden", bufs=2,
                                     name=f"rden{ip}_{unit}")
                    den = npool.tile([1, 1024], F32, tag="den")
                    nc.vector.tensor_copy(den[:], pc[HD:HD + 1, :])
                    nc.vector.reciprocal_approx_fast(den[:], den[:])
                    nc.sync.dma_start(rden[:], den[:])
                    rb = npool.tile([HD, 1024], F32, tag="rb")
                    nc.sync.dma_start(rb[:], rden[0:1, :].to_broadcast((HD, 1024)))
                    if slot == 0:
                        dst = ctxa_sb[HD * unit:HD * (unit + 1), q0:q0 + 1024]
                    else:
                        dst = ctxb_sb[:, q0:q0 + 1024]
                    nc.vector.tensor_tensor(dst, pc[0:HD, :], rb[:], op=ALU.mult)

                if ip == 1:
                    emit_shard(0)
                    emit_shard(1)
                elif ip == 2:
                    emit_shard(2)
                    emit_shard(3)
                    nc.gpsimd.collective_compute(
                        "AllToAll", ALU.bypass,
                        replica_groups=[list(range(NCORES))],
                        ins=[a2a_in.opt()], outs=[a2a_out.opt()],
                    )
            attn_ctx.close()

            # ============== load my batch group's 4 shards, upcast ==============
            with tc.tile_critical():
                with nc.gpsimd.register("qx") as qx_reg:
                    nc.gpsimd.reg_load(qx_reg, qoff_sb[0:1, 0:1])
                    qx_v = nc.gpsimd.snap(qx_reg)

            ctxf_sb = qkv.tile([P, KT, SQ], FP8)
            nc.gpsimd.dma_start(
                ctxf_sb[:],
                a2a_out.rearrange("g d q -> (g d) q")
                .rearrange("(x kt p) q -> p x kt q", p=P, kt=KT)[
                    :, bass.ds(qx_v, 1), :, :],
            )
            ctxg_sb = qkv.tile([P, KT, SQ], BF16)
            for kt in range(KT):
                nc.scalar.activation(ctxg_sb[:, kt, :], ctxf_sb[:, kt, :],
                                     AF.Identity)

            # ================= out dense + residual + LayerNorm =================
            NST = SQ // P
            with tc.tile_pool(name="ops", bufs=2, space="PSUM") as ops, \
                 tc.tile_pool(name="lnp", bufs=1) as lnp:
                h_all = lnp.tile([P, NST, H], F32)
                mu_all = lnp.tile([P, NST], F32)
                var_all = lnp.tile([P, NST], F32)
                sq_tmp = lnp.tile([P, H], F32, bufs=2)
                for st4 in range(NST):
                    ps = ops.tile([P, H], F32, tag="od", name=f"od{st4}")
                    for kt in range(KT):
                        lhsT = ctxg_sb[:, kt, st4 * P:(st4 + 1) * P]
                        nc.tensor.matmul(ps[:, 0:512], lhsT, wo_sb[:, kt, 0:512],
                                         start=(kt == 0), stop=(kt == KT - 1))
                        nc.tensor.matmul(ps[:, 512:H], lhsT, wo_sb[:, kt, 512:H],
                                         start=(kt == 0), stop=(kt == KT - 1))
                    # h = out_dense + (x + bo); mean via ACT copy-accumulate
                    psc = lnp.tile([P, H], F32, tag="psc", bufs=2, name=f"psc{st4}")
                    nc.scalar.activation(psc[:], ps[:], AF.Identity,
                                         accum_out=mu_all[:, st4:st4 + 1])
                    nc.vector.tensor_tensor(h_all[:, st4, :], psc[:],
                                            xres_sb[:, st4, :], op=ALU.add)

                # mu = (psum_rowsum + xres_rowsum)/H ; var via ACT Square-accum
                nc.vector.tensor_tensor(mu_all[:], mu_all[:], xsum_sb[:], op=ALU.add)
                nc.vector.tensor_scalar_mul(mu_all[:], mu_all[:], 1.0 / H)
                negmu = lnp.tile([P, NST], F32)
                nc.vector.tensor_scalar_mul(negmu[:], mu_all[:], -1.0)
                for st4 in range(NST):
                    nc.scalar.activation(sq_tmp[:], h_all[:, st4, :], AF.Square,
                                         bias=negmu[:, st4:st4 + 1],
                                         accum_out=var_all[:, st4:st4 + 1])
                nc.vector.tensor_scalar_mul(var_all[:], var_all[:], 1.0 / H)
                nc.vector.tensor_scalar_add(var_all[:], var_all[:], EPS)
                # rstd = 1/sqrt(var) with one Newton step (batched over stiles)
                std0 = lnp.tile([P, NST], F32)
                nc.scalar.activation(std0[:], var_all[:], AF.Sqrt)
                y0 = lnp.tile([P, NST], F32)
                nc.vector.reciprocal(y0[:], std0[:])
                t0 = lnp.tile([P, NST], F32)
                nc.vector.tensor_tensor(t0[:], y0[:], y0[:], op=ALU.mult)
                nc.vector.tensor_tensor(t0[:], t0[:], var_all[:], op=ALU.mult)
                nc.vector.tensor_scalar_mul(t0[:], t0[:], -0.5)
                nc.vector.tensor_scalar_add(t0[:], t0[:], 1.5)
                rstd = lnp.tile([P, NST], F32)
                nc.vector.tensor_tensor(rstd[:], y0[:], t0[:], op=ALU.mult)

                for st4 in range(NST):
                    hc = lnp.tile([P, H], F32, tag="hc", bufs=2, name=f"hc{st4}")
                    nc.vector.tensor_scalar_sub(hc[:], h_all[:, st4, :],
                                                mu_all[:, st4:st4 + 1])
                    o_sb = lnp.tile([P, H], F32, tag="o", bufs=2, name=f"o{st4}")
                    nc.vector.scalar_tensor_tensor(
                        out=o_sb[:], in0=hc[:], scalar=rstd[:, st4:st4 + 1],
                        in1=lng_sb[:], op0=ALU.mult, op1=ALU.mult)
                    nc.vector.tensor_tensor(o_sb[:], o_sb[:], lnb_sb[:], op=ALU.add)
                    nc.sync.dma_start(out_d[st4 * P:(st4 + 1) * P, :], o_sb[:])

    nc.compile()
    return nc


def _prep_inputs(inputs):
    hs = np.asarray(inputs["hidden_states"], dtype=np.float32)
    am = np.asarray(inputs["attention_mask"], dtype=np.float32)
    Wq = np.asarray(inputs["Wq"], dtype=np.float32)
    Wk = np.asarray(inputs["Wk"], dtype=np.float32)
    Wv = np.asarray(inputs["Wv"], dtype=np.float32)
    Wo = np.asarray(inputs["Wo"], dtype=np.float32)
    bq = np.asarray(inputs["bq"], dtype=np.float32)
    bk = np.asarray(inputs["bk"], dtype=np.float32)
    bv = np.asarray(inputs["bv"], dtype=np.float32)
    bo = np.asarray(inputs["bo"], dtype=np.float32)
    lng = np.asarray(inputs["ln_gamma"], dtype=np.float32)
    lnb = np.asarray(inputs["ln_beta"], dtype=np.float32)

    wo_bf = Wo.astype(ml_dtypes.bfloat16)
    lng_rep = np.ascontiguousarray(np.broadcast_to(lng, (P, H)))
    lnb_rep = np.ascontiguousarray(np.broadcast_to(lnb, (P, H)))

    in_maps = []
    for c in range(NCORES):
        b, g = c // TP, c % TP
        cs = slice(HGD * g, HGD * (g + 1))
        mk = np.where(am[b] >= 0, 0.0, BIGNEG).astype(np.float32)
        mk2 = np.where(am[b] >= 0, np.float32(SCH_B),
                       np.float32(SCH_A * BIGNEG)).astype(np.float32)
        gqv = np.where(am[b] >= 0, 0.0, BIGPOS).astype(ml_dtypes.bfloat16)[None, :]
        in_maps.append({
            "xt": np.ascontiguousarray(hs[b].T).astype(ml_dtypes.bfloat16),
            "xres": np.ascontiguousarray(hs[b, SQ * g:SQ * (g + 1)] + bo),
            "wq": np.ascontiguousarray(Wq[:, cs]).astype(ml_dtypes.bfloat16),
            "wk": np.ascontiguousarray(Wk[:, cs]).astype(ml_dtypes.bfloat16),
            "wv": np.ascontiguousarray(Wv[:, cs]).astype(ml_dtypes.bfloat16),
            "bq": np.ascontiguousarray(bq[cs]),
            "bk": np.ascontiguousarray(bk[cs]),
            "bvr": np.ascontiguousarray(np.broadcast_to(bv[cs], (P, HGD))),
            "wo": np.ascontiguousarray(wo_bf),
            "mkb": mk,
            "mkb2": mk2,
            "gq": np.ascontiguousarray(gqv),
            "lng": lng_rep,
            "lnb": lnb_rep,
            "xsum": np.ascontiguousarray(
                (hs[b, SQ * g:SQ * (g + 1)] + bo).sum(axis=1).astype(np.float32)),
            "qoff": np.array([[b]], dtype=np.uint32),
        })
    return in_maps


def _run(inputs, trace=False, trace_cores=None):
    if "nc" not in _cache:
        _cache["nc"] = build()
    nc = _cache["nc"]
    in_maps = _prep_inputs(inputs)
    res = run_bass_kernel_spmd(
        nc, in_maps, list(range(NCORES)), trace=trace,
        trace_cores=trace_cores,
    )
    out = np.empty((B, S, H), dtype=np.float32)
    for c in range(NCORES):
        b, g = c // TP, c % TP
        out[b, SQ * g:SQ * (g + 1)] = res.results[c]["out"]
    return out, res


def kernel(**inputs) -> np.ndarray:
    out, _ = _run(inputs)
    return out
